# revision 42
# baseline (speedup 1.0000x reference)
"""Causal self-attention (B=2, T=2048, C=1024, H=16) on 8 trn2 NeuronCores.

Sharding: tensor-parallel over heads. Each core owns 2 heads (128 features):
  - qkv projection of the full sequence onto its 384 w_attn columns
  - causal attention for its 2 heads (both batches)
  - partial c_proj: y_local [4096,128] @ w_proj[rows] -> [4096,1024] partial
The 8 partial outputs are summed on the host (the "all-reduce after c_proj"),
plus b_proj.

bf16 pipeline (fp32 PSUM accumulation everywhere): x^T, w, q, k, v, p, y,
w_proj and the output partials are all bf16. Numpy simulation of the full
bf16 pipeline gives rel err ~3.0e-3 vs the fp32 reference (budget 2e-2).

Structure per core:
  - qkv: per 512-token chunk, 3 parts x 8 contraction blocks of
    [128,128]x[128,512] matmuls; V transposed to [kv, feat] via PE transpose.
  - attention: per (batch, 512-q-chunk), kv blocks of 128; S pair for both
    heads row-packed into one [128,1024] psum (concurrent via base-partition
    0/64 row groups); exp on ScalarE (reserved for exp); PV with a 65th
    ones-column in V producing softmax denominators. The PV pipeline lags S
    by 6 blocks and flows continuously ACROSS chunk boundaries so neither
    exp latency nor psY-slot recycling ever stalls the in-order PE stream.
  - normalization: denominator row copy -> reciprocal_approx_fast ->
    partition_broadcast (gpsimd queue carries no big DMAs, so this never
    queues behind DMA-ring waits) -> fused (psum y)*recip -> bf16 yt.
  - proj: [128tok,128feat] stationary x [128,512] w_proj halves; psum->bf16
    copies split across Vector+Scalar for tail chunks; outputs staged per
    chunk and stored with two 512KB batched DMAs on the sync queue.
Scheduling notes (hard-won): Tile derives dependencies from EMISSION order,
so every producer must be emitted before its consumer (ensure_qkv); engine
streams execute in order, so pacing of filler emission controls overlap.
Chunk order (0,0)..(0,3),(1,3)..(1,0) keeps the tail chunk smallest.
"""

import sys

sys.path.insert(0, "/opt/trn_rl_repo")

import numpy as np

N_CORES = 8
B, T, C = 2, 2048, 1024
H, D = 16, 64
HPC = H // N_CORES            # heads per core
F = HPC * D                   # local feature width = 128
BT = B * T                    # 4096 tokens
TCH = 512                     # token chunk (moving-operand width)
NCH = BT // TCH               # 8 token chunks
KB = 128                      # kv block size
NQC = T // TCH                # 4 query chunks per batch

_COMPILED = {}


def _build():
    import concourse.bass as bass
    import concourse.mybir as mybir
    import concourse.tile as tile
    from concourse import bacc

    f32, bf16 = mybir.dt.float32, mybir.dt.bfloat16
    Exp = mybir.ActivationFunctionType.Exp

    nc = bacc.Bacc("TRN2", target_bir_lowering=False, debug=False,
                   num_devices=N_CORES)

    xt = nc.dram_tensor("xt", [C, BT], bf16, kind="ExternalInput")
    wqkv = nc.dram_tensor("wqkv", [C // 128, 128, 3 * F], bf16,
                          kind="ExternalInput")
    bqkv = nc.dram_tensor("bqkv", [F, 3], f32, kind="ExternalInput")
    wp = nc.dram_tensor("wp", [F, C], bf16, kind="ExternalInput")
    tri = nc.dram_tensor("tri", [KB, KB], bf16, kind="ExternalInput")
    eye = nc.dram_tensor("eye", [128, 128], bf16, kind="ExternalInput")
    ones = nc.dram_tensor("ones", [128, 64], bf16, kind="ExternalInput")
    out = nc.dram_tensor("out", [BT, C], bf16, kind="ExternalOutput")

    xt_r = xt.rearrange("(g p) t -> p g t", p=128)     # [128, 8, BT]

    with tile.TileContext(nc) as tc, \
         nc.allow_low_precision(reason="bf16 matmul pipeline, fp32 psum"):
        with tc.tile_pool(name="const", bufs=1) as cpool, \
             tc.tile_pool(name="seq", bufs=1) as seq, \
             tc.tile_pool(name="work", bufs=4) as work, \
             tc.tile_pool(name="psBig", bufs=2, space="PSUM") as psBig, \
             tc.tile_pool(name="psS", bufs=2, space="PSUM") as psS, \
             tc.tile_pool(name="psY", bufs=2, space="PSUM") as psY:

            # ---- resident constants: w_q + chunk-0 x on sync; w_kv rides
            # the otherwise-idle gpsimd queue in parallel ----
            w_sb = cpool.tile([128, C // 128, 3 * F], bf16)
            wqkv_r = wqkv.rearrange("a p f -> p a f")
            nc.sync.dma_start(w_sb[:, :, 0:F], wqkv_r[:, :, 0:F])
            b_sb = cpool.tile([F, 3], f32)
            wp_sb = cpool.tile([F, C], bf16)
            tri_sb = cpool.tile([KB, KB], bf16)
            eye_sb = cpool.tile([128, 128], bf16)
            ones_sb = cpool.tile([128, 64], bf16)

            # ---- resident sequence tensors (per 512-token chunk tiles) ----
            qt_t = [seq.tile([F, TCH], bf16, tag=f"qt{t}", name=f"qt{t}")
                    for t in range(NCH)]
            kt_t = [seq.tile([F, TCH], bf16, tag=f"kt{t}", name=f"kt{t}")
                    for t in range(NCH)]
            # v65[:, i, 0:65] = [V_headA | 1], v65[:, i, 65:130] = [V_headB | 1]
            v65 = seq.tile([128, BT // KB, 130], bf16)
            yt_t = [seq.tile([F, TCH], bf16, tag=f"yt{t}", name=f"yt{t}")
                    for t in range(NCH)]

            def qkv_chunk_gen(t):
                """qkv projection + V transpose for one 512-token chunk.
                Chunk 0 loads x per contraction block on the sync queue (the
                earliest-starting DMA queue) so the first matmul can begin as
                soon as the first 128 rows land."""
                xts = work.tile([128, C // 128, TCH], bf16, tag="xt", bufs=4,
                                name=f"xts{t}")
                if t == 0:
                    for cb in range(8):
                        nc.sync.dma_start(xts[:, cb, :], xt_r[:, cb, 0:TCH])
                else:
                    nc.sync.dma_start(xts[:],
                                      xt_r[:, :, t * TCH:(t + 1) * TCH])
                yield
                for part in range(3):
                    ps = psBig.tile([128, TCH], f32, tag="big",
                                    name=f"pqkv{t}_{part}")
                    for cb in range(8):
                        nc.tensor.matmul(
                            ps[:], w_sb[:, cb, part * F:(part + 1) * F],
                            xts[:, cb, :], start=(cb == 0), stop=(cb == 7))
                        if cb == 3:
                            yield
                    if part == 0:
                        nc.vector.tensor_scalar_add(qt_t[t][:], ps[:],
                                                    b_sb[:, 0:1])
                    elif part == 1:
                        nc.vector.tensor_scalar_add(kt_t[t][:], ps[:],
                                                    b_sb[:, 1:2])
                    else:
                        vt = work.tile([128, TCH], bf16, tag="vt", bufs=2,
                                       name=f"vt{t}")
                        nc.vector.tensor_scalar_add(vt[:], ps[:],
                                                    b_sb[:, 2:3])
                    yield
                ptr = psBig.tile([128, TCH], bf16, tag="big", name=f"ptr{t}")
                for i in range(4):
                    nc.tensor.transpose(ptr[:, i * 128:(i + 1) * 128],
                                        vt[:, i * 128:(i + 1) * 128],
                                        eye_sb[:])
                    if i == 1:
                        yield
                ptr3 = ptr[:].rearrange("p (a k) -> p a k", k=128)
                t4 = t * 4
                nc.vector.tensor_copy(v65[:, t4:t4 + 4, 0:64],
                                      ptr3[:, :, 0:64])
                nc.vector.tensor_copy(v65[:, t4:t4 + 4, 65:129],
                                      ptr3[:, :, 64:128])
                yield

            class Filler:
                """Paced emission of qkv/proj work into attention gaps.
                qkv chunk generators are tracked individually so a chunk can
                be force-drained right before attention first needs it."""

                def __init__(self):
                    self.qkv = {}
                    self.others = []

                def add_qkv(self, t, g):
                    self.qkv[t] = g

                def add(self, g):
                    self.others.append(g)

                def ensure_qkv(self, t):
                    for tt in sorted(self.qkv):
                        if tt > t:
                            break
                        for _ in self.qkv.pop(tt):
                            pass

                def step(self):
                    for tt in sorted(self.qkv):
                        try:
                            next(self.qkv[tt])
                            return
                        except StopIteration:
                            del self.qkv[tt]
                    while self.others:
                        try:
                            next(self.others[0])
                            return
                        except StopIteration:
                            self.others.pop(0)

                def drain(self):
                    self.ensure_qkv(NCH)
                    while self.others:
                        for _ in self.others.pop(0):
                            pass

            def attn_pair(b, bq, bk):
                """S for both heads into one [128,1024] psum tile + one exp.
                Returns the P tile (halves = heads)."""
                qchunk = b * NQC + bq
                kchunk = b * NQC + bk // 4
                kcol = (bk % 4) * 128
                r = bk - 4 * bq
                trim = 128 * r if r > 0 else 0
                s_ps = psS.tile([128, 2 * TCH], f32, tag="s",
                                name=f"s{b}{bq}{bk}")
                for h in range(HPC):
                    hs = h * 64
                    nc.tensor.matmul(
                        s_ps[:, h * TCH + trim:(h + 1) * TCH],
                        kt_t[kchunk][hs:hs + 64, kcol:kcol + 128],
                        qt_t[qchunk][hs:hs + 64, trim:],
                        start=True, stop=True)
                p_t = work.tile([128, 2 * TCH], bf16, tag="p", bufs=10,
                                name=f"p{b}{bq}{bk}")
                if r <= 0:
                    nc.scalar.activation(p_t[:], s_ps[:], Exp)
                else:
                    s3 = s_ps[:].rearrange("p (a q) -> p a q", a=2)
                    p3 = p_t[:].rearrange("p (a q) -> p a q", a=2)
                    nc.scalar.activation(p3[:, :, trim:], s3[:, :, trim:], Exp)
                if r >= 0:
                    for h in range(HPC):
                        c0 = h * TCH + trim
                        nc.vector.tensor_mul(p_t[:, c0:c0 + 128],
                                             p_t[:, c0:c0 + 128], tri_sb[:])
                return p_t

            pend = []           # PV pipeline, continuous ACROSS chunks

            def pend_pop(fl):
                emit_pv, bk, p_t, stop, norm_fn, _ = pend.pop(0)
                emit_pv(bk, p_t, stop)
                if norm_fn is not None:
                    norm_fn()
                    fl.step()

            def attention_chunk(b, bq, fl, steps=1):
                qchunk = b * NQC + bq
                nblk = 4 * bq + 4
                yt_ps = [psY.tile([65, TCH], f32, tag="yt",
                                  name=f"ytps{b}{bq}{h}")
                         for h in range(HPC)]

                def emit_pv(bk, p_t, stop):
                    vti = b * (T // KB) + bk
                    r = bk - 4 * bq
                    trim = 128 * r if r > 0 else 0
                    for h in range(HPC):
                        nc.tensor.matmul(
                            yt_ps[h][:, trim:],
                            v65[:, vti, 65 * h:65 * h + 65],
                            p_t[:, h * TCH + trim:(h + 1) * TCH],
                            start=(bk == 0), stop=stop)

                def norm_fn():
                    # normalization: copy the psum denominator row to SBUF,
                    # fast-reciprocal it, partition-broadcast (the gpsimd
                    # queue is kept free of big DMAs so this never queues
                    # behind DMA-ring waits), fused (psum y)*recip -> bf16 yt.
                    # proj is released HERE to keep producer-before-consumer
                    # emission order (Tile derives deps from emission order).
                    for h in range(HPC):
                        hs = h * 64
                        dst = work.tile([1, TCH], f32, tag="dst", bufs=2,
                                        name=f"dst{b}{bq}{h}")
                        nc.vector.tensor_copy(dst[:], yt_ps[h][64:65, :])
                        rec = work.tile([1, TCH], f32, tag="rec", bufs=2,
                                        name=f"rec{b}{bq}{h}")
                        nc.vector.reciprocal_approx_fast(rec[:], dst[:])
                        bca = work.tile([128, TCH], f32, tag="bca", bufs=2,
                                        name=f"bca{b}{bq}{h}")
                        nc.gpsimd.partition_broadcast(bca[:], rec[:])
                        nc.vector.tensor_mul(yt_t[qchunk][hs:hs + 64, :],
                                             yt_ps[h][0:64, :],
                                             bca[hs:hs + 64, :])
                    fl.add(proj_gen(b, bq, late=(qchunk in (4, 5, 6))))

                # producers must be EMITTED before consumers: Tile derives
                # dependencies from emission order, so force-drain the qkv
                # generators this chunk reads before emitting any S matmul.
                # The PV pipeline lags S by 6 blocks and flows across chunk
                # boundaries so the exp chain and psY recycling never stall
                # the in-order PE stream.
                fl.ensure_qkv(qchunk)
                for bk in range(nblk):
                    p_t = attn_pair(b, bq, bk)
                    # drain the previous chunk's tail at 2/block so its norm
                    # (and psY release) lands several blocks before this
                    # chunk's first PV allocates the recycled psY slots
                    if pend and pend[0][5] != qchunk:
                        pend_pop(fl)
                        if pend and pend[0][5] != qchunk and len(pend) >= 3:
                            pend_pop(fl)
                    if len(pend) >= 6:
                        pend_pop(fl)
                    pend.append((emit_pv, bk, p_t, bk == nblk - 1,
                                 norm_fn if bk == nblk - 1 else None, qchunk))
                    for _ in range(steps):
                        fl.step()

            def proj_gen(b, bq, late):
                """Projection of one chunk + batched output stores. Late
                chunks split the psum->bf16 copies across Scalar (idle after
                exp ends) and Vector; early ones keep ScalarE free for exp."""
                qchunk = b * NQC + bq
                ost = work.tile([128, NQC, C], bf16, tag="ost", bufs=3,
                                name=f"ost{qchunk}")
                for ic in range(4):
                    for cc in range(2):
                        pj = psBig.tile([128, TCH], f32, tag="big",
                                        name=f"pj{qchunk}_{ic}_{cc}")
                        nc.tensor.matmul(
                            pj[:],
                            yt_t[qchunk][:, ic * 128:(ic + 1) * 128],
                            wp_sb[:, cc * TCH:(cc + 1) * TCH],
                            start=True, stop=True)
                        if late and (ic + cc) % 2 == 0:
                            nc.scalar.copy(
                                ost[:, ic, cc * TCH:(cc + 1) * TCH], pj[:])
                        else:
                            nc.vector.tensor_copy(
                                ost[:, ic, cc * TCH:(cc + 1) * TCH], pj[:])
                        yield
                    if ic % 2 == 1:
                        lo = ic - 1
                        r0 = qchunk * TCH + lo * 128
                        dst = out[r0:r0 + 256, :].rearrange(
                            "(a p) c -> p a c", p=128)
                        nc.sync.dma_start(dst, ost[:, lo:ic + 1, :])
                        yield

            def chain(*gens):
                for g in gens:
                    yield from g

            # ---- schedule ----
            g0 = qkv_chunk_gen(0)
            next(g0)            # after w_q: chunk-0 per-block x loads (sync)
            nc.sync.dma_start(b_sb[:], bqkv[:])
            nc.gpsimd.dma_start(w_sb[:, :, F:3 * F], wqkv_r[:, :, F:3 * F])
            nc.gpsimd.dma_start(tri_sb[:], tri[:])
            nc.gpsimd.dma_start(eye_sb[:], eye[:])
            nc.gpsimd.dma_start(ones_sb[:], ones[:])
            nc.gpsimd.dma_start(wp_sb[:], wp[:])
            nc.vector.tensor_copy(
                v65[:, :, 64::65],
                ones_sb[:, 0:64].rearrange("p (a b) -> p a b", b=2))
            for _ in g0:
                pass
            fl = Filler()
            for t in range(1, NCH):
                fl.add_qkv(t, qkv_chunk_gen(t))
            order = [(0, 0), (0, 1), (0, 2), (0, 3),
                     (1, 3), (1, 2), (1, 1), (1, 0)]
            for i, (b, bq) in enumerate(order):
                attention_chunk(b, bq, fl, steps=3 if i >= 6 else 2)
            while pend:
                pend_pop(fl)
                fl.step()
                fl.step()
            fl.drain()
    nc.compile()
    return nc


def _get_nc():
    if "nc" not in _COMPILED:
        _COMPILED["nc"] = _build()
    return _COMPILED["nc"]


def _prep_in_maps(x, w_attn, b_attn, w_proj):
    import ml_dtypes
    bf = ml_dtypes.bfloat16

    x = np.asarray(x, np.float32)
    w_attn = np.asarray(w_attn, np.float32)
    b_attn = np.asarray(b_attn, np.float32)
    w_proj = np.asarray(w_proj, np.float32)

    scale = np.float32(1.0 / np.sqrt(D))
    xt = np.ascontiguousarray(x.reshape(BT, C).T.astype(bf))   # [C, BT]
    # tri[kv, j] = 1 when j >= kv (upper triangular incl diagonal)
    tri = np.ascontiguousarray(np.triu(np.ones((KB, KB), np.float32))).astype(bf)
    eye = np.eye(128, dtype=np.float32).astype(bf)
    ones = np.ones((128, 64), bf)

    in_maps = []
    for c in range(N_CORES):
        cols = slice(c * F, (c + 1) * F)
        wq = w_attn[:, cols] * scale
        wk = w_attn[:, C + c * F:C + (c + 1) * F]
        wv = w_attn[:, 2 * C + c * F:2 * C + (c + 1) * F]
        wqkv = np.ascontiguousarray(
            np.concatenate([wq, wk, wv], axis=1)
            .reshape(C // 128, 128, 3 * F)).astype(bf)
        bq = b_attn[c * F:(c + 1) * F] * scale
        bk = b_attn[C + c * F:C + (c + 1) * F]
        bv = b_attn[2 * C + c * F:2 * C + (c + 1) * F]
        bqkv = np.ascontiguousarray(np.stack([bq, bk, bv], axis=1))
        wpc = np.ascontiguousarray(w_proj[c * F:(c + 1) * F, :]).astype(bf)
        in_maps.append({
            "xt": xt, "wqkv": wqkv, "bqkv": bqkv, "wp": wpc,
            "tri": tri, "eye": eye, "ones": ones,
        })
    return in_maps


def _run(inputs, trace=False):
    from concourse.bass_utils import run_bass_kernel_spmd

    nc = _get_nc()
    in_maps = _prep_in_maps(inputs["x"], inputs["w_attn"], inputs["b_attn"],
                            inputs["w_proj"])
    res = run_bass_kernel_spmd(nc, in_maps, list(range(N_CORES)), trace=trace)
    b_proj = np.asarray(inputs["b_proj"], np.float32)
    acc = np.zeros((BT, C), np.float64)
    for c in range(N_CORES):
        acc += res.results[c]["out"].astype(np.float32)
    y = (acc + b_proj).astype(np.float32).reshape(B, T, C)
    return y, res


def kernel(**inputs):
    y, _ = _run(inputs, trace=False)
    return y
